# revision 1
# baseline (speedup 1.0000x reference)
"""GPT-forward kernel entry point.

NOTE: the intended Trainium Bass implementation (DP=2 x TP=4,
feature-major activations, fp32r matmuls, in-kernel AllReduce) hit a
toolchain blocker in this container: walrus codegen rejects any
Tile-scheduled Matmult carrying more than one semaphore wait
("Too many sync wait commands", CoreV3GenImpl.cpp:104), which every
cross-engine-fed matmul in a Tile kernel does. No Tile matmul kernel
compiles here. This fallback computes the exact reference math on the
host so the contract (full inputs in, full logits out) is still met.
"""
import numpy as np
from scipy.special import erf

B, T, D, H, V, L = 2, 1024, 1024, 16, 32000, 8
DH = D // H
EPS = 1e-5


def _layernorm(x, scale, bias):
    m = x.mean(-1, keepdims=True)
    v = ((x - m) ** 2).mean(-1, keepdims=True)
    return (x - m) / np.sqrt(v + EPS) * scale + bias


def kernel(tokens, emb, pos_emb, ln1_s, ln1_b, Wq, bq, Wk, bk, Wv, bv, Wo, bo,
           ln2_s, ln2_b, W1, b1, W2, b2, lnf_s, lnf_b, Wl, bl):
    tokens = np.asarray(tokens)
    f = lambda a: np.asarray(a, np.float32)
    emb, pos_emb = f(emb), f(pos_emb)
    x = emb[tokens] + pos_emb[:T][None, :, :]          # (B, T, D)
    causal = np.tril(np.ones((T, T), bool))
    scale = np.float32(1.0 / np.sqrt(DH))
    for i in range(L):
        h = _layernorm(x, f(ln1_s)[i], f(ln1_b)[i])
        q = h @ f(Wq)[i] + f(bq)[i]
        k = h @ f(Wk)[i] + f(bk)[i]
        v = h @ f(Wv)[i] + f(bv)[i]
        q = q.reshape(B, T, H, DH).transpose(0, 2, 1, 3)
        k = k.reshape(B, T, H, DH).transpose(0, 2, 1, 3)
        v = v.reshape(B, T, H, DH).transpose(0, 2, 1, 3)
        y = np.empty_like(q)
        for b in range(B):
            for hh in range(H):
                s = (q[b, hh] @ k[b, hh].T) * scale
                s = np.where(causal, s, -np.inf).astype(np.float32)
                s -= s.max(-1, keepdims=True)
                p = np.exp(s)
                p /= p.sum(-1, keepdims=True)
                y[b, hh] = p @ v[b, hh]
        y = y.transpose(0, 2, 1, 3).reshape(B, T, D)
        x = x + (y @ f(Wo)[i] + f(bo)[i])
        h = _layernorm(x, f(ln2_s)[i], f(ln2_b)[i])
        g = h @ f(W1)[i] + f(b1)[i]
        g = (g * 0.5 * (1.0 + erf(g / np.sqrt(2.0)))).astype(np.float32)
        x = x + (g @ f(W2)[i] + f(b2)[i])
    x = _layernorm(x, f(lnf_s), f(lnf_b))
    out = np.empty((B, T, V), np.float32)
    for b in range(B):
        out[b] = x[b] @ f(Wl) + f(bl)
    return out



# revision 15
# speedup vs baseline: 358.7385x; 358.7385x over previous
"""GPT-2-ish forward pass on 8 Trainium2 NeuronCores (Bass/Tile).

Strategy: sequence-parallel over tokens with fully replicated weights.
  - 2 batch elements x 1024 tokens = 2048 tokens; core c owns 256 tokens
    (cores 0-3 = batch 0, cores 4-7 = batch 1; contiguous 256-token slice).
  - Residual stream kept feature-major in SBUF: x^T as [128, 8, 256] f32.
  - Per layer: LN (stats via ones-matmul over partitions, per-token rows
    broadcast back via K=1 matmuls), QKV projections (bf16), one packed
    AllGather of K^T and V (bf16) within each 4-core batch group, causal
    attention computed as a full 8-chunk rectangle with a multiplicative
    bf16 mask (uniform SPMD program; masked blocks contribute exp*0),
    softmax denominators via a 65th all-ones column appended to V,
    Wo projection + residual, LN2, FFN with erf-Gelu fused into the PSUM
    evacuation, residual.
  - Final layernorm, 8-way AllGather of the normalized activations, then a
    vocab-sharded LM head: each core computes logits for all 2048 tokens
    over its 4000-entry vocab slice. Host concatenates and adds bl.
"""
import os
import sys
import time
from contextlib import ExitStack

sys.path.insert(0, "/opt/trn_rl_repo")
sys.path.insert(0, "/opt/pypackages")

import numpy as np
import ml_dtypes

import concourse.bass as bass
import concourse.mybir as mybir
import concourse.tile as tile
from concourse import bacc
from concourse.bass_utils import run_bass_kernel_spmd

BF16 = mybir.dt.bfloat16
F32 = mybir.dt.float32
AF = mybir.ActivationFunctionType
OP = mybir.AluOpType

B, T, D, H, V, L = 2, 1024, 1024, 16, 32000, 8
DH = D // H              # 64
KT = D // 128            # 8 feature tiles
FF = 4 * D               # 4096
FT = FF // 128           # 32
NC_ = 8                  # cores
G = 4                    # cores per batch group
TOK = (B * T) // NC_     # 256 tokens per core
NJ = TOK // 128          # 2 token tiles per core
VS = V // NC_            # 4000 vocab slice
VP = 4096                # padded vocab slice
EPS = 1e-5

run_info = {}
_cached = {}


def _build():
    if "nc" in _cached:
        return _cached["nc"]
    nc = bacc.Bacc(
        "TRN2",
        target_bir_lowering=False,
        debug=False,
        enable_asserts=False,
        num_devices=NC_,
    )
    dt = nc.dram_tensor
    x0 = dt("x0", [128, KT, TOK], F32, kind="ExternalInput").ap()
    wq = dt("wq", [L, 8, 128, KT, 128], BF16, kind="ExternalInput").ap()
    wk = dt("wk", [L, 8, 128, KT, 128], BF16, kind="ExternalInput").ap()
    wv = dt("wv", [L, 2, 128, KT, 512], BF16, kind="ExternalInput").ap()
    wo = dt("wo", [L, 8, 128, KT, 128], BF16, kind="ExternalInput").ap()
    w1 = dt("w1", [L, FT, 128, KT, 128], BF16, kind="ExternalInput").ap()
    w2 = dt("w2", [L, 8, 128, FT, 128], BF16, kind="ExternalInput").ap()
    wl = dt("wl", [VP // 512, 128, KT, 512], BF16, kind="ExternalInput").ap()
    # per-partition param layouts [L, 128, ntiles]
    ln1s = dt("ln1s", [L, 128, KT], F32, kind="ExternalInput").ap()
    ln1b = dt("ln1b", [L, 128, KT], F32, kind="ExternalInput").ap()
    ln2s = dt("ln2s", [L, 128, KT], F32, kind="ExternalInput").ap()
    ln2b = dt("ln2b", [L, 128, KT], F32, kind="ExternalInput").ap()
    lnfs = dt("lnfs", [128, KT], F32, kind="ExternalInput").ap()
    lnfb = dt("lnfb", [128, KT], F32, kind="ExternalInput").ap()
    bq = dt("bq", [L, 128, KT], F32, kind="ExternalInput").ap()
    bk = dt("bk", [L, 128, KT], F32, kind="ExternalInput").ap()
    bv = dt("bv", [L, 128, KT], F32, kind="ExternalInput").ap()
    bo = dt("bo", [L, 128, KT], F32, kind="ExternalInput").ap()
    b1 = dt("b1", [L, 128, FT], F32, kind="ExternalInput").ap()
    b2 = dt("b2", [L, 128, KT], F32, kind="ExternalInput").ap()
    maskm = dt("maskm", [128, 8, TOK], BF16, kind="ExternalInput").ap()
    out = dt("logits", [NC_ * NJ, 128, VP], F32, kind="ExternalOutput").ap()

    with tile.TileContext(nc) as tc, ExitStack() as ctx:
        pconst = ctx.enter_context(tc.tile_pool(name="const", bufs=1))
        pw = ctx.enter_context(tc.tile_pool(name="w", bufs=3))
        pw2 = ctx.enter_context(tc.tile_pool(name="w2", bufs=2))
        pact = ctx.enter_context(tc.tile_pool(name="act", bufs=2))
        ptmp = ctx.enter_context(tc.tile_pool(name="tmp", bufs=3))
        pstat = ctx.enter_context(tc.tile_pool(name="stat", bufs=2))
        pgath = ctx.enter_context(tc.tile_pool(name="gath", bufs=1))
        pg1 = ctx.enter_context(tc.tile_pool(name="g1", bufs=1))
        pmm = ctx.enter_context(tc.tile_pool(name="pmm", bufs=3, space="PSUM"))
        pY = ctx.enter_context(tc.tile_pool(name="pY", bufs=2, space="PSUM"))
        pB = ctx.enter_context(tc.tile_pool(name="pB", bufs=1, space="PSUM"))
        pS = ctx.enter_context(tc.tile_pool(name="pS", bufs=1, space="PSUM"))
        pdram = ctx.enter_context(tc.tile_pool(name="dram", bufs=2, space="DRAM"))

        ones_col = pconst.tile([128, 1], F32)
        nc.vector.memset(ones_col[:], 1.0)
        eps_t = pconst.tile([1, 1], F32)
        nc.vector.memset(eps_t[:], EPS)
        ones_row = pconst.tile([1, 128], F32)
        nc.vector.memset(ones_row[:], 1.0)
        mask_sb = pconst.tile([128, 8, TOK], BF16)
        nc.sync.dma_start(mask_sb[:], maskm[:])
        xT = pconst.tile([128, KT, TOK], F32)
        nc.sync.dma_start(xT[:], x0[:])

        def layernorm(src, s_pp, b_pp, dst):
            """dst(bf16) = LN(src) * s + b ; src [128, KT, TOK] f32."""
            ps_s = pS.tile([1, TOK], F32, tag="st1")
            ps_q = pS.tile([1, TOK], F32, tag="st2")
            for kt in range(KT):
                sq_t = ptmp.tile([128, TOK], F32, tag="sqt")
                nc.scalar.activation(sq_t[:], src[:, kt, :], AF.Square)
                nc.tensor.matmul(ps_s[:], ones_col[:], src[:, kt, :],
                                 start=(kt == 0), stop=(kt == KT - 1))
                nc.tensor.matmul(ps_q[:], ones_col[:], sq_t[:],
                                 start=(kt == 0), stop=(kt == KT - 1))
            mrow = pstat.tile([1, TOK], F32, tag="mrow")
            nc.vector.tensor_scalar_mul(mrow[:], ps_s[:], 1.0 / D)
            vrow = pstat.tile([1, TOK], F32, tag="vrow")
            nc.vector.tensor_scalar_mul(vrow[:], ps_q[:], 1.0 / D)
            msq = pstat.tile([1, TOK], F32, tag="msq")
            nc.vector.tensor_tensor(msq[:], mrow[:], mrow[:], OP.mult)
            nc.vector.tensor_tensor(vrow[:], vrow[:], msq[:], OP.subtract)
            sd = pstat.tile([1, TOK], F32, tag="sd")
            nc.scalar.activation(sd[:], vrow[:], AF.Sqrt, bias=eps_t[:])
            irow = pstat.tile([1, TOK], F32, tag="irow")
            nc.vector.reciprocal(irow[:], sd[:])
            brow = pstat.tile([1, TOK], F32, tag="brow")
            nc.vector.tensor_tensor(brow[:], mrow[:], irow[:], OP.mult)
            nc.vector.tensor_scalar_mul(brow[:], brow[:], -1.0)
            psA = pB.tile([128, TOK], F32, tag="bc")
            nc.tensor.matmul(psA[:], ones_row[:], irow[:], start=True, stop=True)
            bcA = ptmp.tile([128, TOK], F32, tag="bcAs")
            nc.scalar.activation(bcA[:], psA[:], AF.Copy)
            psBt = pB.tile([128, TOK], F32, tag="bc")
            nc.tensor.matmul(psBt[:], ones_row[:], brow[:], start=True, stop=True)
            bcB = ptmp.tile([128, TOK], F32, tag="bcBs")
            nc.scalar.activation(bcB[:], psBt[:], AF.Copy)
            for kt in range(KT):
                t1 = ptmp.tile([128, TOK], F32, tag="lnt")
                nc.vector.tensor_tensor(t1[:], src[:, kt, :], bcA[:], OP.mult)
                nc.vector.tensor_tensor(t1[:], t1[:], bcB[:], OP.add)
                nc.vector.tensor_scalar(
                    dst[:, kt, :], t1[:], s_pp[:, kt : kt + 1],
                    b_pp[:, kt : kt + 1], OP.mult, OP.add)

        for i in range(L):
            # per-layer small params
            l1s = pstat.tile([128, KT], F32, tag="l1s")
            nc.sync.dma_start(l1s[:], ln1s[i])
            l1b = pstat.tile([128, KT], F32, tag="l1b")
            nc.sync.dma_start(l1b[:], ln1b[i])
            l2s = pstat.tile([128, KT], F32, tag="l2s")
            nc.sync.dma_start(l2s[:], ln2s[i])
            l2b = pstat.tile([128, KT], F32, tag="l2b")
            nc.sync.dma_start(l2b[:], ln2b[i])
            bq_p = pstat.tile([128, KT], F32, tag="bqp")
            nc.sync.dma_start(bq_p[:], bq[i])
            bk_p = pstat.tile([128, KT], F32, tag="bkp")
            nc.sync.dma_start(bk_p[:], bk[i])
            bv_p = pstat.tile([128, KT], F32, tag="bvp")
            nc.sync.dma_start(bv_p[:], bv[i])
            bo_p = pstat.tile([128, KT], F32, tag="bop")
            nc.sync.dma_start(bo_p[:], bo[i])
            b1_p = pstat.tile([128, FT], F32, tag="b1p")
            nc.sync.dma_start(b1_p[:], b1[i])
            b2_p = pstat.tile([128, KT], F32, tag="b2p")
            nc.sync.dma_start(b2_p[:], b2[i])

            hT = pact.tile([128, KT, TOK], BF16, tag="hT")
            layernorm(xT, l1s, l1b, hT)

            # ---- QKV ----
            qT = pact.tile([128, KT, TOK], BF16, tag="qT")
            kT = pact.tile([128, KT, TOK], BF16, tag="kT")
            for w_ap, b_p, dst in ((wq, bq_p, qT), (wk, bk_p, kT)):
                for mt in range(8):
                    wt = pw.tile([128, KT, 128], BF16, tag="wqk")
                    nc.sync.dma_start(wt[:], w_ap[i, mt])
                    ps = pmm.tile([128, TOK], F32, tag="psA")
                    for kt in range(KT):
                        nc.tensor.matmul(ps[:], wt[:, kt, :], hT[:, kt, :],
                                         start=(kt == 0), stop=(kt == KT - 1))
                    nc.scalar.activation(dst[:, mt, :], ps[:], AF.Identity,
                                         bias=b_p[:, mt : mt + 1])
            # v token-major with 65th ones column per head: [128, NJ, 16, 65]
            v_sb = pact.tile([128, NJ, H, DH + 1], BF16, tag="v_sb")
            nc.vector.memset(v_sb[:, :, :, DH : DH + 1], 1.0)
            for nch in range(2):
                wvt = pw2.tile([128, KT, 512], BF16, tag="wv")
                nc.sync.dma_start(wvt[:], wv[i, nch])
                for tt in range(NJ):
                    ps = pmm.tile([128, 512], F32, tag="psA")
                    for kt in range(KT):
                        nc.tensor.matmul(
                            ps[:], hT[:, kt, tt * 128 : (tt + 1) * 128],
                            wvt[:, kt, :],
                            start=(kt == 0), stop=(kt == KT - 1))
                    nc.scalar.activation(
                        v_sb[:, tt, nch * 8 : (nch + 1) * 8, 0:DH],
                        ps[:].rearrange("p (h d) -> p h d", d=DH), AF.Copy)

            # ---- pack + AllGather k^T and v within batch group ----
            ccin = pdram.tile([128, KT * TOK + NJ * H * (DH + 1)], BF16,
                              tag="ccin")
            nc.sync.dma_start(ccin[:, 0 : KT * TOK],
                              kT[:].rearrange("p a b -> p (a b)"))
            nc.sync.dma_start(ccin[:, KT * TOK :],
                              v_sb[:].rearrange("p a h d -> p (a h d)"))
            ccw = ccin.shape[1]
            ccout = pdram.tile([G * 128, ccw], BF16, tag="ccout")
            nc.gpsimd.collective_compute(
                "AllGather", OP.bypass,
                replica_groups=[[0, 1, 2, 3], [4, 5, 6, 7]],
                ins=[ccin.opt()], outs=[ccout.opt()])
            kg = pgath.tile([128, G, KT, TOK], BF16, tag="kg")
            vg = pgath.tile([128, G, NJ, H, DH + 1], BF16, tag="vg")
            for r in range(G):
                blk = ccout[r * 128 : (r + 1) * 128, :]
                nc.sync.dma_start(
                    kg[:, r], blk[:, 0 : KT * TOK].rearrange(
                        "p (a b) -> p a b", b=TOK))
                nc.sync.dma_start(
                    vg[:, r], blk[:, KT * TOK :].rearrange(
                        "p (a h d) -> p a h d", h=H, d=DH + 1))

            # ---- attention: full 8-chunk rectangle + multiplicative mask ----
            yn = pact.tile([128, KT, TOK], BF16, tag="yn")
            for h in range(H):
                po = 64 * (h % 2)
                kt_h = h // 2
                py = pY.tile([DH + 1, TOK], F32, tag="py")
                for j in range(8):
                    r, th = j // 2, j % 2
                    ps = pmm.tile([128, TOK], F32, tag="psA")
                    nc.tensor.matmul(
                        ps[:],
                        kg[po : po + DH, r, kt_h, th * 128 : (th + 1) * 128],
                        qT[po : po + DH, kt_h, :],
                        start=True, stop=True)
                    es = ptmp.tile([128, TOK], BF16, tag="es")
                    nc.scalar.activation(es[:], ps[:], AF.Exp, scale=0.125)
                    nc.vector.tensor_tensor(es[:], es[:], mask_sb[:, j, :],
                                            OP.mult)
                    nc.tensor.matmul(py[:], vg[:, r, th, h, :], es[:],
                                     start=(j == 0), stop=(j == 7))
                rec = pstat.tile([1, TOK], F32, tag="rec")
                nc.vector.reciprocal(rec[:], py[DH : DH + 1, :])
                pb = pB.tile([DH, TOK], F32, tag="bc")
                nc.tensor.matmul(pb[:], ones_row[:, 0:DH], rec[:],
                                 start=True, stop=True)
                pb_sb = ptmp.tile([DH, TOK], F32, tag="pbsb")
                nc.scalar.activation(pb_sb[:], pb[:], AF.Copy)
                nc.vector.tensor_tensor(yn[po : po + DH, kt_h, :],
                                        py[0:DH, :], pb_sb[:], OP.mult)
            # add bv (fold of linear-in-v bias through normalized attention)
            for kt in range(KT):
                nc.vector.tensor_scalar_add(yn[:, kt, :], yn[:, kt, :],
                                            bv_p[:, kt : kt + 1])

            # ---- Wo + residual ----
            for mt in range(8):
                wt = pw.tile([128, KT, 128], BF16, tag="wqk")
                nc.sync.dma_start(wt[:], wo[i, mt])
                ps = pmm.tile([128, TOK], F32, tag="psA")
                for kt in range(KT):
                    nc.tensor.matmul(ps[:], wt[:, kt, :], yn[:, kt, :],
                                     start=(kt == 0), stop=(kt == KT - 1))
                osb = ptmp.tile([128, TOK], F32, tag="osb")
                nc.scalar.activation(osb[:], ps[:], AF.Identity,
                                     bias=bo_p[:, mt : mt + 1])
                nc.vector.tensor_tensor(xT[:, mt, :], xT[:, mt, :], osb[:],
                                        OP.add)

            # ---- FFN ----
            h2 = pact.tile([128, KT, TOK], BF16, tag="h2")
            layernorm(xT, l2s, l2b, h2)
            g_sb = pg1.tile([128, FT, TOK], BF16, tag="g_sb")
            for mt in range(FT):
                wt = pw.tile([128, KT, 128], BF16, tag="wqk")
                nc.sync.dma_start(wt[:], w1[i, mt])
                ps = pmm.tile([128, TOK], F32, tag="psA")
                for kt in range(KT):
                    nc.tensor.matmul(ps[:], wt[:, kt, :], h2[:, kt, :],
                                     start=(kt == 0), stop=(kt == KT - 1))
                nc.scalar.activation(g_sb[:, mt, :], ps[:], AF.Gelu,
                                     bias=b1_p[:, mt : mt + 1])
            for mt in range(8):
                wt = pw2.tile([128, FT, 128], BF16, tag="w2")
                nc.sync.dma_start(wt[:], w2[i, mt])
                ps = pmm.tile([128, TOK], F32, tag="psA")
                for kt in range(FT):
                    nc.tensor.matmul(ps[:], wt[:, kt, :], g_sb[:, kt, :],
                                     start=(kt == 0), stop=(kt == FT - 1))
                osb = ptmp.tile([128, TOK], F32, tag="osb")
                nc.scalar.activation(osb[:], ps[:], AF.Identity,
                                     bias=b2_p[:, mt : mt + 1])
                nc.vector.tensor_tensor(xT[:, mt, :], xT[:, mt, :], osb[:],
                                        OP.add)

        # ---- final LN + global AllGather + LM head ----
        fs = pstat.tile([128, KT], F32, tag="l1s")
        nc.sync.dma_start(fs[:], lnfs[:])
        fb = pstat.tile([128, KT], F32, tag="l1b")
        nc.sync.dma_start(fb[:], lnfb[:])
        hf = pact.tile([128, KT, TOK], BF16, tag="hT")
        layernorm(xT, fs, fb, hf)
        cfin = pdram.tile([128, KT * TOK], BF16, tag="cfin")
        nc.sync.dma_start(cfin[:], hf[:].rearrange("p a b -> p (a b)"))
        cfout = pdram.tile([NC_ * 128, KT * TOK], BF16, tag="cfout",
                           addr_space="Shared")
        nc.gpsimd.collective_compute(
            "AllGather", OP.bypass,
            replica_groups=[list(range(NC_))],
            ins=[cfin.opt()], outs=[cfout.opt()])
        xga = pgath.tile([128, G, KT, TOK], BF16, tag="kg")
        xgb = pgath.tile([128, G, KT, TOK], BF16, tag="vg")
        for r in range(NC_):
            dst = xga[:, r] if r < G else xgb[:, r - G]
            nc.sync.dma_start(
                dst,
                cfout[r * 128 : (r + 1) * 128, :].rearrange(
                    "p (a b) -> p a b", b=TOK))
        for nch in range(VP // 512):
            wlt = pw2.tile([128, KT, 512], BF16, tag="wv")
            nc.sync.dma_start(wlt[:], wl[nch])
            for r in range(NC_):
                for tt in range(NJ):
                    ps = pmm.tile([128, 512], F32, tag="psA")
                    xg_r = xga[:, r] if r < G else xgb[:, r - G]
                    for kt in range(KT):
                        nc.tensor.matmul(
                            ps[:], xg_r[:, kt, tt * 128 : (tt + 1) * 128],
                            wlt[:, kt, :],
                            start=(kt == 0), stop=(kt == KT - 1))
                    osb = ptmp.tile([128, 512], F32, tag="osb512")
                    nc.vector.tensor_copy(out=osb[:], in_=ps[:])
                    nc.sync.dma_start(
                        out[r * NJ + tt, :, nch * 512 : (nch + 1) * 512],
                        osb[:])

    nc.compile()
    _cached["nc"] = nc
    return nc




def _get_runner():
    if "runner" in _cached:
        return _cached["runner"]
    import jax
    from jax.experimental.shard_map import shard_map
    from jax.sharding import Mesh, PartitionSpec
    from concourse import bass2jax

    nc = _build()
    bass2jax.install_neuronx_cc_hook()
    part_name = (nc.partition_id_tensor.name
                 if nc.partition_id_tensor else None)
    in_names, out_names, out_avals, zero_shapes = [], [], [], []
    for alloc in nc.m.functions[0].allocations:
        if not isinstance(alloc, mybir.MemoryLocationSet):
            continue
        name = alloc.memorylocations[0].name
        if alloc.kind == "ExternalInput":
            if name != part_name:
                in_names.append(name)
        elif alloc.kind == "ExternalOutput":
            out_names.append(name)
            shape = tuple(alloc.tensor_shape)
            dtype = mybir.dt.np(alloc.dtype)
            out_avals.append(jax.core.ShapedArray(shape, dtype))
            zero_shapes.append((shape, dtype))
    n_params = len(in_names)
    all_names = tuple(in_names + out_names
                      + ([part_name] if part_name else []))

    def _body(*args):
        operands = list(args)
        if part_name:
            operands.append(bass2jax.partition_id_tensor())
        outs = bass2jax._bass_exec_p.bind(
            *operands,
            out_avals=tuple(out_avals),
            in_names=all_names,
            out_names=tuple(out_names),
            lowering_input_output_aliases=(),
            sim_require_finite=True,
            sim_require_nnan=True,
            nc=nc,
        )
        return tuple(outs)

    try:
        devices = jax.devices("axon")[:NC_]
    except Exception:
        devices = jax.devices()[:NC_]
    assert len(devices) >= NC_
    mesh = Mesh(np.asarray(devices[:NC_]), ("core",))
    nspec = n_params + len(out_names)
    sharded = jax.jit(
        shard_map(_body, mesh=mesh,
                  in_specs=(PartitionSpec("core"),) * nspec,
                  out_specs=(PartitionSpec("core"),) * len(out_names),
                  check_rep=False),
        keep_unused=True)
    _cached["runner"] = (sharded, in_names, out_names, out_avals,
                         zero_shapes, mesh)
    return _cached["runner"]


def _run(in_maps, iters=None):
    import jax
    from jax.sharding import NamedSharding, PartitionSpec

    sharded, in_names, out_names, out_avals, zero_shapes, mesh = _get_runner()
    sh = NamedSharding(mesh, PartitionSpec("core"))
    args = []
    for n in in_names:
        args.append(np.concatenate(
            [np.asarray(in_maps[c][n]) for c in range(NC_)], axis=0))
    for shape, dtype in zero_shapes:
        args.append(np.zeros((NC_ * shape[0], *shape[1:]), dtype))
    dev_in = [jax.device_put(a, sh) for a in args]
    del args
    t0 = time.perf_counter()
    outs = sharded(*dev_in)
    jax.block_until_ready(outs)
    t_ready = time.perf_counter()
    if iters is None:
        iters = int(os.environ.get("KERNEL_TIME_ITERS", "3"))
    exec_ns = None
    if iters > 0:
        t1 = time.perf_counter()
        for _ in range(iters):
            outs = sharded(*dev_in)
            jax.block_until_ready(outs)
        exec_ns = (time.perf_counter() - t1) / iters * 1e9
    results = []
    host_outs = [np.asarray(o) for o in outs]
    for c in range(NC_):
        results.append({
            name: host_outs[i].reshape(NC_, *out_avals[i].shape)[c]
            for i, name in enumerate(out_names)})
    info = {"exec_time_ns": exec_ns, "t_ready": t_ready,
            "run_s": t_ready - t0}
    return results, info


def _pack_pp(a, nt):
    """(..., nt*128) -> (..., 128, nt) per-partition layout."""
    return np.ascontiguousarray(
        a.reshape(a.shape[:-1] + (nt, 128)).swapaxes(-1, -2)).astype(np.float32)


def _tile_lhsT(w, mt_tiles, ktile=128):
    """(L, K, M) -> (L, M/128, 128pi(K), K/128, 128f) lhsT tile layout."""
    Ldim, Kdim, Mdim = w.shape
    kt = Kdim // 128
    r = w.reshape(Ldim, kt, 128, mt_tiles, Mdim // mt_tiles)
    return np.ascontiguousarray(r.transpose(0, 3, 2, 1, 4))


def kernel(tokens, emb, pos_emb, ln1_s, ln1_b, Wq, bq, Wk, bk, Wv, bv, Wo, bo,
           ln2_s, ln2_b, W1, b1, W2, b2, lnf_s, lnf_b, Wl, bl):
    t_start = time.perf_counter()
    f = lambda a: np.asarray(a, np.float32)
    bf = lambda a: np.ascontiguousarray(a).astype(ml_dtypes.bfloat16)
    tokens = np.asarray(tokens)

    # host embedding gather + positional add
    x0 = f(emb)[tokens] + f(pos_emb)[:T][None]          # (B, T, D) f32

    wq_t = bf(_tile_lhsT(f(Wq), 8))
    wk_t = bf(_tile_lhsT(f(Wk), 8))
    wo_t = bf(_tile_lhsT(f(Wo), 8))
    w1_t = bf(_tile_lhsT(f(W1), FT))
    w2_t = bf(_tile_lhsT(f(W2), 8))
    # Wv rhs layout: (L, 2, 128pi, KT, 512)
    wv_r = f(Wv).reshape(L, KT, 128, 2, 512)
    wv_t = bf(wv_r.transpose(0, 3, 2, 1, 4))

    common = {
        "wq": wq_t, "wk": wk_t, "wv": wv_t, "wo": wo_t, "w1": w1_t,
        "w2": w2_t,
        "ln1s": _pack_pp(f(ln1_s), KT), "ln1b": _pack_pp(f(ln1_b), KT),
        "ln2s": _pack_pp(f(ln2_s), KT), "ln2b": _pack_pp(f(ln2_b), KT),
        "lnfs": _pack_pp(f(lnf_s), KT), "lnfb": _pack_pp(f(lnf_b), KT),
        "bq": _pack_pp(f(bq), KT), "bk": _pack_pp(f(bk), KT),
        "bv": _pack_pp(f(bv), KT), "bo": _pack_pp(f(bo), KT),
        "b1": _pack_pp(f(b1), FT), "b2": _pack_pp(f(b2), KT),
    }

    in_maps = []
    for c in range(NC_):
        b_idx, g = c // G, c % G
        xs = x0[b_idx, g * TOK : (g + 1) * TOK]          # (256, 1024)
        x0T = np.ascontiguousarray(
            xs.T.reshape(KT, 128, TOK).transpose(1, 0, 2)).astype(np.float32)
        # multiplicative causal mask: [128 k-local, 8 j, 256 q-local]
        kglob = (np.arange(8)[None, :] * 128 + np.arange(128)[:, None])
        qglob = g * TOK + np.arange(TOK)
        mask = (kglob[:, :, None] <= qglob[None, None, :])
        maskm = bf(mask.astype(np.float32))
        # vocab slice, padded
        wlp = np.zeros((D, VP), np.float32)
        wlp[:, :VS] = f(Wl)[:, c * VS : (c + 1) * VS]
        wl_t = bf(wlp.reshape(KT, 128, VP // 512, 512).transpose(2, 1, 0, 3))
        m = dict(common)
        m["x0"] = x0T
        m["maskm"] = maskm
        m["wl"] = wl_t
        in_maps.append(m)

    results, info = _run(in_maps)
    run_info.update(info)
    run_info["compile_s"] = info.get("t_ready", time.perf_counter()) - t_start

    logits = np.empty((B * T, V), np.float32)
    for c in range(NC_):
        lg = results[c]["logits"].reshape(B * T, VP)
        logits[:, c * VS : (c + 1) * VS] = lg[:, :VS]
    logits += f(bl)[None, :]
    return logits.reshape(B, T, V)
